# revision 40
# baseline (speedup 1.0000x reference)
"""BERT-CRF loss kernel for Trainium2 (8 NeuronCores, data-parallel over batch).

Computation: emissions = x @ W.T + b; CRF NLL = mean over batch of
(denominator log-partition - numerator tag-path score).

v5 strategy per core (2 sequences, 8192 time steps):
  Sharding/layout: each core receives its batch shard pre-transposed AND
  pre-cast to bf16 as xT [768, 8192] (h-major), so the h-contraction lands on
  the partition dim directly (no on-device transposes/casts) and the HBM
  stream is halved to 12.6 MB (bf16 emissions were already validated by the
  original baseline at 3.5e-05 rel err, ~500x inside the 2e-2 gate).

  Stage 1: tapered DMA blocks (1-group at the edges for latency, 2-group in
  the middle) stream xT on both hardware DGE queues (SP + ACT, pre-issued
  ahead of compute so transfers never serialize behind dependent copies);
  per 512-t group, 6 accumulating bf16 matmuls produce e[3, 512] in PSUM;
  a 16-matmul PE warm-up during the startup window keeps the tensor engine
  DVFS-ramped so real matmuls run ~380ns.  PSUM stages to SBUF [3, 2048]
  and redistributes (3 small DMAs per 4-group batch, on the otherwise-idle
  SP queue; the last batch splits in two for earlier tail start) into
  per-half tiles e_sb[h][p, c, u]: partition p holds the 32 consecutive
  time steps t = 4096*h + 32*p + u.

  Stage 2 (CRF denominator): forward algorithm as a chain of log-semiring
  products of 3x3 matrices M_t[i,j] = trans[i,j] + b[j] + e_t[j].  Each
  partition pair-combines its 32 matrices once (level 1, 32 -> 16), fused
  via a host-precomputed K[i,k,j] = ct[i,j] + ct[j,k] table and a
  per-timestep-max rescale that removes the max-reduce.  All of sequence 0
  and 3/4 of sequence 1 run interleaved with stage 1; only the last
  32-partition slice is an exposed tail.  The 16 partial products per
  partition ship to the host, which finishes each sequence's product in
  float64 (vectorized tree over 2048 tiny 3x3 log-matmuls per sequence).

  Numerator: e * one-hot(y) multiply + free-dim reduce per half gives
  sum_t e[t, y_t] per partition; host sums and adds start/end/transition/bias
  path scores (tiny O(B*S) int gathers, as in torchcrf's score decomposition).

Assumes mask == all-ones (guaranteed by the problem spec: fill "ones").
"""

import sys

sys.path.insert(0, "/opt/trn_rl_repo")

import numpy as np
import ml_dtypes
from contextlib import ExitStack

import concourse.bass as bass
import concourse.mybir as mybir
import concourse.tile as tile
from concourse.bass_utils import run_bass_kernel_spmd

dt = mybir.dt
AF = mybir.ActivationFunctionType
ALU = mybir.AluOpType
AX = mybir.AxisListType

# ---------------------------------------------------------------------------
# The walrus build in this container accepts at most ONE sync wait per
# instruction.  Legalize the serialized BIR by moving extra waits onto
# preceding same-engine NoOps (each carrying exactly one wait).
# ---------------------------------------------------------------------------
_orig_to_json_bytes = bass.Bass.to_json_bytes


def _legalized_to_json_bytes(self):
    import json as _json

    m = _json.loads(_orig_to_json_bytes(self))
    ctr = 0
    for fn in m.get("functions", []):
        for blk in fn.get("blocks", []):
            insts = blk.get("instructions", [])
            out = []
            for inst in insts:
                si = inst.get("sync_info") or {}
                waits = si.get("on_wait") or []
                if len(waits) > 1:
                    for w in waits[:-1]:
                        ctr += 1
                        out.append(
                            {
                                "debug": inst.get("debug", 0),
                                "engine": inst["engine"],
                                "ins": [],
                                "outs": [],
                                "name": f"lw-{ctr}",
                                "opcode": "NoOp",
                                "sync_info": {"on_update": [], "on_wait": [w]},
                            }
                        )
                    si["on_wait"] = [waits[-1]]
                out.append(inst)
            blk["instructions"] = out
    return _json.dumps(m).encode()


bass.Bass.to_json_bytes = _legalized_to_json_bytes

B, S, H, T = 16, 4096, 768, 3
NCORES = 8
BL = B // NCORES          # sequences per core = 2
NT = BL * S               # 8192 time steps per core
NGROUP = 16               # groups of 512 time steps
HC = H // 128             # 6 h-chunks
UP = 32                   # time steps per partition (short chains)
M1 = UP // 2              # level-1 pairs per partition = 16
L2M = M1 // 2             # matrices shipped to host per partition = 8

_CACHE = {}


def _build_program():
    nc = bass.Bass()
    tc = tile.TileContext(nc)

    # ---- DRAM I/O ----
    xt_d = nc.dram_tensor("xt", [H, NT], dt.bfloat16, kind="ExternalInput")
    wt_d = nc.dram_tensor("wt", [128, HC * T], dt.bfloat16, kind="ExternalInput")
    cf_d = nc.dram_tensor("cf", [128, 54 + 2 * T * UP], dt.float32,
                          kind="ExternalInput")
    op_d = nc.dram_tensor("op", [128, 2 * M1 * 9 + 2], dt.float32,
                          kind="ExternalOutput")

    with tc, ExitStack() as ctx:
        const_pool = ctx.enter_context(tc.tile_pool(name="const", bufs=1))
        xg_pool = ctx.enter_context(tc.tile_pool(name="xg", bufs=4))
        st_pool = ctx.enter_context(tc.tile_pool(name="st", bufs=2))
        e_pool = ctx.enter_context(tc.tile_pool(name="e", bufs=1))
        scr_pool = ctx.enter_context(tc.tile_pool(name="scr", bufs=1))
        ps_e_pool = ctx.enter_context(tc.tile_pool(name="pse", bufs=4, space="PSUM"))

        # ---- constants (issued after the first xT block DMAs) ----
        wt_sb = const_pool.tile([128, HC * T], dt.bfloat16, tag="wt")
        cf_sb = const_pool.tile([128, 54 + 2 * T * UP], dt.float32, tag="cf")
        k1_v = cf_sb[:, 0:27].rearrange("p (ik j) -> p ik j", j=3)
        k0_v = cf_sb[:, 27:54].rearrange("p (ik j) -> p ik j", j=3)

        # per-half emission tiles: e_sb[h][p, c, u], partition p holds the
        # 32 consecutive time steps t = 4096*h + 32*p + u
        e_sb = [
            e_pool.tile([128, T * UP], dt.float32, tag=f"e{h}", name=f"e{h}")
            for h in range(2)
        ]
        # outputs staging: 2*72 tree results + 2 numerator columns
        op_st = e_pool.tile([128, 2 * M1 * 9 + 2], dt.float32, tag="opst")

        def emit_tree(half, part, plo=0, phi=128):
            """Emit one chunk of the in-partition tree for one half.
            part 0: rescale prep + L1 S-build; part 1: L1 finish;
            part 2: L2 + write into op_st.  Parts 0/1 may be emitted for a
            64-aligned partition slice [plo:phi] to overlap stage 1."""
            np_ = phi - plo
            e3 = e_sb[half][:].rearrange("p (c u) -> p c u", u=UP)[plo:phi]
            if part == 0:
                if plo == 0:
                    _CACHE[f"tree{half}"] = (
                        scr_pool.tile([128, UP], dt.float32, tag="emax",
                                      name=f"emax{half}"),
                        scr_pool.tile([128, T * UP], dt.float32, tag="es",
                                      name=f"es{half}"),
                        scr_pool.tile([128, M1 * 27], dt.float32, tag="s1",
                                      name=f"s1_{half}"),
                    )
                emax, es_t, s1 = _CACHE[f"tree{half}"]
                emx = emax[plo:phi]
                # emax[p,u] = max_c e[p,c,u];  es = e - emax (range <= 0)
                nc.vector.tensor_tensor(
                    emx, e3[:, 0, :], e3[:, 1, :], op=ALU.max
                )
                nc.vector.tensor_tensor(
                    emx, emx, e3[:, 2, :], op=ALU.max
                )
                emax_b = emx.unsqueeze(1).broadcast_to([np_, T, UP])
                nc.vector.tensor_tensor(
                    es_t[:].rearrange("p (c u) -> p c u", u=UP)[plo:phi], e3,
                    emax_b, op=ALU.subtract,
                )
                # L1 S-build: S[p,m,ik,j] = K[ik,j] + esA[m,j]
                es3 = es_t[:].rearrange("p (c u) -> p c u", u=UP)[plo:phi]
                esA = es3.rearrange("p c (m two) -> p m two c", two=2)
                s4 = s1[:].rearrange("p (m ik j) -> p m ik j", ik=9, j=3)[plo:phi]
                a1 = esA[:, 1:, 0, :].unsqueeze(2).broadcast_to(
                    [np_, M1 - 1, 9, 3]
                )
                nc.vector.tensor_tensor(
                    s4[:, 1:, :, :],
                    k1_v[plo:phi].unsqueeze(1).broadcast_to([np_, M1 - 1, 9, 3]),
                    a1, op=ALU.add,
                )
                a0 = esA[:, 0:1, 0, :].unsqueeze(2).broadcast_to([np_, 1, 9, 3])
                nc.vector.tensor_tensor(
                    s4[:, 0:1, :, :],
                    k0_v[plo:phi].unsqueeze(1).broadcast_to([np_, 1, 9, 3]),
                    a0, op=ALU.add,
                )
            elif part == 1:
                emax, es_t, s1 = _CACHE[f"tree{half}"]
                es3 = es_t[:].rearrange("p (c u) -> p c u", u=UP)[plo:phi]
                if plo == 0:
                    _CACHE[f"tree{half}b"] = (
                        scr_pool.tile([128, M1 * 9], dt.float32, tag="sm1",
                                      name=f"sm1_{half}"),
                        scr_pool.tile([128, M1 * 9], dt.float32, tag="c1",
                                      name=f"c1_{half}"),
                        scr_pool.tile([128, M1], dt.float32, tag="ems",
                                      name=f"ems{half}"),
                    )
                sm1, c1, emsum = _CACHE[f"tree{half}b"]
                nc.scalar.activation(s1[plo:phi], s1[plo:phi], AF.Exp)
                nc.vector.tensor_reduce(
                    sm1[plo:phi],
                    s1[:].rearrange("p (g j) -> p g j", j=3)[plo:phi],
                    axis=AX.X, op=ALU.add,
                )
                nc.scalar.activation(sm1[plo:phi], sm1[plo:phi], AF.Ln)
                # C1 = ln-sum + esB[k] + (emaxA + emaxB)  (= lnsum + eB + emaxA)
                esB = (
                    es3.rearrange("p c (m two) -> p m two c", two=2)[:, :, 1, :]
                    .unsqueeze(2)
                    .broadcast_to([np_, M1, 3, 3])
                )                                                  # [p,m,i0,k]
                c14 = c1[:].rearrange("p (m i k) -> p m i k", i=3, k=3)[plo:phi]
                nc.vector.tensor_tensor(
                    c14,
                    sm1[:].rearrange("p (m i k) -> p m i k", i=3, k=3)[plo:phi],
                    esB, op=ALU.add,
                )
                em2 = emax[:].rearrange("p (m two) -> p m two", two=2)[plo:phi]
                nc.vector.tensor_tensor(
                    emsum[plo:phi], em2[:, :, 0], em2[:, :, 1], op=ALU.add
                )
                em_b = (
                    emsum[plo:phi].unsqueeze(2).unsqueeze(3)
                    .broadcast_to([np_, M1, 3, 3])
                )
                opv = op_st[:, 144 * half : 144 * (half + 1)].rearrange(
                    "p (m i k) -> p m i k", i=3, k=3
                )[plo:phi]
                nc.vector.tensor_tensor(opv, c14, em_b, op=ALU.add)
            elif part == 2:
                # L2 pair-combine: c1 [128, 16, 9] -> op_st 8 matrices
                c1 = _CACHE[f"tree{half}b"][1]
                nm = L2M
                s_t = scr_pool.tile([128, nm * 27], dt.float32, tag="s2",
                                    name=f"s2_{half}")
                mx_t = scr_pool.tile([128, nm * 9], dt.float32, tag="mx2",
                                     name=f"mx2_{half}")
                sm_t = scr_pool.tile([128, nm * 9], dt.float32, tag="sm2",
                                     name=f"sm2_{half}")
                c_out = op_st[:, 72 * half : 72 * (half + 1)]
                cv = c1[:].rearrange("p (m two e) -> p m two e", two=2, e=9)
                b_kj = cv[:, :, 1, :].rearrange("p m (j k) -> p m k j", k=3)
                s5 = s_t[:].rearrange("p (m i k j) -> p m i k j", i=3, k=3, j=3)
                for i in range(3):
                    a_i = (
                        cv[:, :, 0, 3 * i : 3 * i + 3]
                        .unsqueeze(2)
                        .broadcast_to([128, nm, 3, 3])
                    )
                    nc.vector.tensor_tensor(
                        s5[:, :, i, :, :], a_i, b_kj, op=ALU.add
                    )
                s3 = s_t[:].rearrange("p (g j) -> p g j", j=3)
                nc.vector.tensor_reduce(mx_t[:], s3, axis=AX.X, op=ALU.max)
                mx_b = mx_t[:].unsqueeze(2).broadcast_to([128, nm * 9, 3])
                nc.vector.tensor_tensor(s3, s3, mx_b, op=ALU.subtract)
                nc.scalar.activation(s_t[:], s_t[:], AF.Exp)
                nc.vector.tensor_reduce(sm_t[:], s3, axis=AX.X, op=ALU.add)
                nc.scalar.activation(sm_t[:], sm_t[:], AF.Ln)
                nc.vector.tensor_tensor(c_out, sm_t[:], mx_t[:], op=ALU.add)

        def emit_numerator(half, plo=0, phi=128):
            yoh = cf_sb[:, 54 + T * UP * half : 54 + T * UP * (half + 1)]
            if plo == 0:
                _CACHE[f"nsc{half}"] = scr_pool.tile(
                    [128, T * UP], dt.float32, tag="nsc", name=f"nsc{half}"
                )
            scr = _CACHE[f"nsc{half}"]
            nc.vector.tensor_tensor(
                scr[plo:phi], e_sb[half][plo:phi], yoh[plo:phi], op=ALU.mult
            )
            nc.vector.tensor_reduce(
                op_st[plo:phi, 288 + half : 289 + half], scr[plo:phi],
                axis=AX.X, op=ALU.add,
            )

        # ---- stage 1 + interleaved tree emission ----
        # block list: single-group blocks at the edges for lower pipeline
        # latency, 2-group blocks in the middle for best queue throughput
        BLOCKS = [(0,), (1,), (2, 3), (4, 5), (6, 7), (8, 9), (10, 11),
                  (12, 13), (14,), (15,)]
        first_of = {blk[0]: (i, blk) for i, blk in enumerate(BLOCKS)}

        def issue_block(ib):
            blk = BLOCKS[ib]
            n = len(blk)
            xg = xg_pool.tile([128, HC * 512 * n], dt.bfloat16, tag="xg",
                              name=f"xgb{ib % 4}_{n}")
            xq = nc.sync if ib % 2 == 0 else nc.scalar
            xq.dma_start(
                xg[:].rearrange("p (j t) -> p j t", t=512 * n),
                xt_d[:]
                .rearrange("(j p) t -> p j t", p=128)[:, :, 512 * blk[0] :
                                                     512 * (blk[-1] + 1)],
            )
            return xg

        xgs = {}
        xgs[0] = issue_block(0)
        xgs[1] = issue_block(1)
        nc.sync.dma_start(wt_sb[:], wt_d[:])
        nc.sync.dma_start(cf_sb[:], cf_d[:])
        xgs[2] = issue_block(2)
        xgs[3] = issue_block(3)
        # PE warm-up during the startup window: keeps the tensor engine's
        # DVFS ramped so real matmuls run at full clock from the first block
        wu_ps = ps_e_pool.tile([T, 512], dt.float32, tag="wups", bufs=1)
        for w in range(16):
            nc.tensor.matmul(
                wu_ps[:],
                wt_sb[:, 0:T],
                wt_sb[:, 0:1].broadcast_to([128, 512]),
                start=(w == 0),
                stop=(w == 15),
            )
        for g in range(NGROUP):
            if g in first_of:
                ib, blk = first_of[g]
                if ib + 4 < len(BLOCKS):
                    xgs[ib + 4] = issue_block(ib + 4)
                xg = xgs.pop(ib)
                goff = g
            e_ps = ps_e_pool.tile([T, 512], dt.float32, tag="eps")
            sg = g - goff
            nblk = xg.shape[1] // (HC * 512)
            for j in range(HC):
                nc.tensor.matmul(
                    e_ps[:],
                    wt_sb[:, T * j : T * (j + 1)],
                    xg[:, 512 * nblk * j + 512 * sg : 512 * nblk * j
                       + 512 * (sg + 1)],
                    start=(j == 0),
                    stop=(j == HC - 1),
                )
            if g % 4 == 0:
                e_stage = st_pool.tile([T, 2048], dt.float32, tag="estage")
            nc.scalar.activation(
                e_stage[:, 512 * (g % 4) : 512 * (g % 4 + 1)], e_ps[:], AF.Copy
            )
            if g in (3, 7, 11):
                # redistribute 4 groups (2048 t = 64 rows of 32 steps)
                h, q = g // 8, (g // 4) % 2
                for c in range(T):
                    nc.sync.dma_start(
                        e_sb[h][:].rearrange("p (c u) -> p c u", u=UP)[
                            64 * q : 64 * (q + 1), c
                        ],
                        e_stage[c : c + 1, :].rearrange("z (r u) -> z r u", u=UP),
                    )
            elif g in (13, 15):
                # last batch redistributes in 2-group chunks so the final
                # tree slices can start earlier
                r0 = 64 + 32 * ((g - 13) // 2)
                co = 1024 * ((g - 13) // 2)
                for c in range(T):
                    nc.sync.dma_start(
                        e_sb[1][:].rearrange("p (c u) -> p c u", u=UP)[
                            r0 : r0 + 32, c
                        ],
                        e_stage[c : c + 1, co : co + 1024].rearrange(
                            "z (r u) -> z r u", u=UP
                        ),
                    )
            # interleave trees: all of half 0 and the first 64 partitions of
            # half 1 hide under stage 1; the tail is only L1[64:128] + L2
            if g == 8:
                emit_tree(0, 0, 0, 64)
                emit_tree(0, 1, 0, 64)
            elif g == 9:
                emit_tree(0, 0, 64, 128)
                emit_tree(0, 1, 64, 128)
            elif g == 11:
                emit_numerator(0)
            elif g == 12:
                emit_tree(1, 0, 0, 64)
            elif g == 13:
                emit_tree(1, 1, 0, 64)
            elif g == 14:
                emit_numerator(1, 0, 64)

        # ---- tail: second-half tree remainder + numerator + output ----
        emit_tree(1, 0, 64, 128)
        emit_tree(1, 1, 64, 128)
        emit_numerator(1, 64, 128)
        nc.sync.dma_start(op_d[:], op_st[:])

    return nc


def _get_program():
    if "nc" not in _CACHE:
        _CACHE["nc"] = _build_program()
    return _CACHE["nc"]


def kernel(x, y, mask, W, b, start_transitions, end_transitions, transitions):
    x = np.asarray(x, dtype=np.float32)
    y = np.asarray(y, dtype=np.int32)
    W = np.asarray(W, dtype=np.float32)
    b = np.asarray(b, dtype=np.float32)
    start_t = np.asarray(start_transitions, dtype=np.float32)
    end_t = np.asarray(end_transitions, dtype=np.float32)
    trans = np.asarray(transitions, dtype=np.float32)

    nc = _get_program()

    # ---- host-prepared constants ----
    wt = np.zeros((128, HC * T), dtype=np.float32)
    for j in range(HC):
        for c in range(T):
            wt[:, T * j + c] = W[c, 128 * j : 128 * (j + 1)]

    ct = trans + b[None, :]                      # ct[i,j] = trans[i,j]+b[j]
    k1 = np.empty((3, 3, 3), dtype=np.float32)   # k1[i,k,j] = ct[i,j]+ct[j,k]
    k0 = np.empty((3, 3, 3), dtype=np.float32)   # alpha0 row: start[j]+b[j]+ct[j,k]
    sb = start_t + b
    for i in range(3):
        for k in range(3):
            for j in range(3):
                k1[i, k, j] = ct[i, j] + ct[j, k]
                k0[i, k, j] = sb[j] + ct[j, k]
    cf_base = np.zeros((128, 54), dtype=np.float32)
    cf_base[:, 0:27] = k1.reshape(27)[None, :]
    cf_base[:, 27:54] = k1.reshape(27)[None, :]
    cf_base[0, 27:54] = k0.reshape(27)

    in_maps = []
    for core in range(NCORES):
        b0 = BL * core
        yc = y[b0 : b0 + BL].reshape(2, 128, UP)           # (h, p, u)
        yoh = np.zeros((128, 2, T, UP), dtype=np.float32)  # (p, h, c, u)
        for c in range(T):
            yoh[:, :, c, :] = (yc == c).transpose(1, 0, 2)
        cf = np.concatenate([cf_base, yoh.reshape(128, 2 * T * UP)], axis=1)
        im = {
            "xt": np.ascontiguousarray(
                x[b0 : b0 + BL].reshape(NT, H).T
            ).astype(ml_dtypes.bfloat16),
            "wt": wt.astype(ml_dtypes.bfloat16),
            "cf": np.ascontiguousarray(cf),
        }
        in_maps.append(im)

    _CACHE["last_in_maps"] = in_maps
    res = run_bass_kernel_spmd(nc, in_maps, core_ids=list(range(NCORES)))
    results = res.results

    # ---- host epilogue ----
    chains = np.empty((B, 128 * M1, 3, 3), dtype=np.float64)
    gsum = np.empty(B, dtype=np.float64)
    for core in range(NCORES):
        op = np.asarray(results[core]["op"], dtype=np.float64)  # [128, 290]
        for h in range(BL):
            bidx = BL * core + h
            chains[bidx] = op[:, 144 * h : 144 * (h + 1)].reshape(128 * M1, 3, 3)
            gsum[bidx] = op[:, 288 + h].sum()

    # vectorized log-semiring product over the chain (float64)
    cur = chains
    while cur.shape[1] > 1:
        A = cur[:, 0::2]                                   # [B, n, 3, 3] (i,j)
        Bm = cur[:, 1::2]                                  # [B, n, 3, 3] (j,k)
        s = A[:, :, :, :, None] + Bm[:, :, None, :, :]     # [B, n, i, j, k]
        m = s.max(axis=3)
        cur = m + np.log(np.exp(s - m[:, :, :, None, :]).sum(axis=3))
    P = cur[:, 0]                                          # [B, 3, 3]

    losses = np.zeros(B, dtype=np.float64)
    for bidx in range(B):
        yb = y[bidx]
        az = P[bidx, 0, :] + end_t.astype(np.float64)
        mz = az.max()
        denom = mz + np.log(np.exp(az - mz).sum())
        num = (
            start_t[yb[0]]
            + gsum[bidx]
            + b[yb].sum()                     # bias not in device emissions
            + trans[yb[:-1], yb[1:]].sum()
            + end_t[yb[-1]]
        )
        losses[bidx] = num - denom
    return np.float32(-np.mean(losses))


# revision 41
# speedup vs baseline: 1.1554x; 1.1554x over previous
"""BERT-CRF loss kernel for Trainium2 (8 NeuronCores, data-parallel over batch).

Computation: emissions = x @ W.T + b; CRF NLL = mean over batch of
(denominator log-partition - numerator tag-path score).

v5 strategy per core (2 sequences, 8192 time steps):
  Sharding/layout: each core receives its batch shard pre-transposed AND
  pre-cast to bf16 as xT [768, 8192] (h-major), so the h-contraction lands on
  the partition dim directly (no on-device transposes/casts) and the HBM
  stream is halved to 12.6 MB (bf16 emissions were already validated by the
  original baseline at 3.5e-05 rel err, ~500x inside the 2e-2 gate).

  Stage 1: tapered DMA blocks (1-group at the edges for latency, 2-group in
  the middle) stream xT on both hardware DGE queues (SP + ACT, pre-issued
  ahead of compute so transfers never serialize behind dependent copies);
  per 512-t group, 6 accumulating bf16 matmuls produce e[3, 512] in PSUM;
  a 16-matmul PE warm-up during the startup window keeps the tensor engine
  DVFS-ramped so real matmuls run ~380ns.  PSUM stages to SBUF [3, 2048]
  and redistributes (3 small DMAs per 4-group batch, on the otherwise-idle
  SP queue; the last batch splits in two for earlier tail start) into
  per-half tiles e_sb[h][p, c, u]: partition p holds the 32 consecutive
  time steps t = 4096*h + 32*p + u.

  Stage 2 (CRF denominator): forward algorithm as a chain of log-semiring
  products of 3x3 matrices M_t[i,j] = trans[i,j] + b[j] + e_t[j].  Each
  partition pair-combines its 32 matrices once (level 1, 32 -> 16), fused
  via a host-precomputed K[i,k,j] = ct[i,j] + ct[j,k] table and a
  per-timestep-max rescale that removes the max-reduce.  All of sequence 0
  and 3/4 of sequence 1 run interleaved with stage 1; only the last
  32-partition slice is an exposed tail.  The 16 partial products per
  partition ship to the host, which finishes each sequence's product in
  float64 (vectorized tree over 2048 tiny 3x3 log-matmuls per sequence).

  Numerator: e * one-hot(y) multiply + free-dim reduce per half gives
  sum_t e[t, y_t] per partition; host sums and adds start/end/transition/bias
  path scores (tiny O(B*S) int gathers, as in torchcrf's score decomposition).

Assumes mask == all-ones (guaranteed by the problem spec: fill "ones").
"""

import sys

sys.path.insert(0, "/opt/trn_rl_repo")

import numpy as np
import ml_dtypes
from contextlib import ExitStack

import concourse.bass as bass
import concourse.mybir as mybir
import concourse.tile as tile
from concourse.bass_utils import run_bass_kernel_spmd

dt = mybir.dt
AF = mybir.ActivationFunctionType
ALU = mybir.AluOpType
AX = mybir.AxisListType

# ---------------------------------------------------------------------------
# The walrus build in this container accepts at most ONE sync wait per
# instruction.  Legalize the serialized BIR by moving extra waits onto
# preceding same-engine NoOps (each carrying exactly one wait).
# ---------------------------------------------------------------------------
_orig_to_json_bytes = bass.Bass.to_json_bytes


def _legalized_to_json_bytes(self):
    import json as _json

    m = _json.loads(_orig_to_json_bytes(self))
    ctr = 0
    for fn in m.get("functions", []):
        for blk in fn.get("blocks", []):
            insts = blk.get("instructions", [])
            out = []
            for inst in insts:
                si = inst.get("sync_info") or {}
                waits = si.get("on_wait") or []
                if len(waits) > 1:
                    for w in waits[:-1]:
                        ctr += 1
                        out.append(
                            {
                                "debug": inst.get("debug", 0),
                                "engine": inst["engine"],
                                "ins": [],
                                "outs": [],
                                "name": f"lw-{ctr}",
                                "opcode": "NoOp",
                                "sync_info": {"on_update": [], "on_wait": [w]},
                            }
                        )
                    si["on_wait"] = [waits[-1]]
                out.append(inst)
            blk["instructions"] = out
    return _json.dumps(m).encode()


bass.Bass.to_json_bytes = _legalized_to_json_bytes

B, S, H, T = 16, 4096, 768, 3
NCORES = 8
BL = B // NCORES          # sequences per core = 2
NT = BL * S               # 8192 time steps per core
NGROUP = 16               # groups of 512 time steps
HC = H // 128             # 6 h-chunks
UP = 32                   # time steps per partition (short chains)
M1 = UP // 2              # level-1 pairs per partition = 16
L2M = M1 // 2             # matrices shipped to host per partition = 8

_CACHE = {}


def _build_program():
    nc = bass.Bass()
    tc = tile.TileContext(nc)

    # ---- DRAM I/O ----
    xt_d = nc.dram_tensor("xt", [H, NT], dt.bfloat16, kind="ExternalInput")
    wt_d = nc.dram_tensor("wt", [128, HC * T], dt.bfloat16, kind="ExternalInput")
    cf_d = nc.dram_tensor("cf", [128, 54 + 2 * T * UP], dt.float32,
                          kind="ExternalInput")
    op_d = nc.dram_tensor("op", [128, 2 * M1 * 9 + 2], dt.float32,
                          kind="ExternalOutput")

    with tc, ExitStack() as ctx:
        const_pool = ctx.enter_context(tc.tile_pool(name="const", bufs=1))
        xg_pool = ctx.enter_context(tc.tile_pool(name="xg", bufs=4))
        st_pool = ctx.enter_context(tc.tile_pool(name="st", bufs=2))
        e_pool = ctx.enter_context(tc.tile_pool(name="e", bufs=1))
        scr_pool = ctx.enter_context(tc.tile_pool(name="scr", bufs=1))
        ps_e_pool = ctx.enter_context(tc.tile_pool(name="pse", bufs=4, space="PSUM"))

        # ---- constants (issued after the first xT block DMAs) ----
        wt_sb = const_pool.tile([128, HC * T], dt.bfloat16, tag="wt")
        cf_sb = const_pool.tile([128, 54 + 2 * T * UP], dt.float32, tag="cf")
        k1_v = cf_sb[:, 0:27].rearrange("p (ik j) -> p ik j", j=3)
        k0_v = cf_sb[:, 27:54].rearrange("p (ik j) -> p ik j", j=3)

        # per-half emission tiles: e_sb[h][p, c, u], partition p holds the
        # 32 consecutive time steps t = 4096*h + 32*p + u
        e_sb = [
            e_pool.tile([128, T * UP], dt.float32, tag=f"e{h}", name=f"e{h}")
            for h in range(2)
        ]
        # outputs staging: 2*72 tree results + 2 numerator columns
        op_st = e_pool.tile([128, 2 * M1 * 9 + 2], dt.float32, tag="opst")

        def emit_tree(half, part, plo=0, phi=128):
            """Emit one chunk of the in-partition tree for one half.
            part 0: rescale prep + L1 S-build; part 1: L1 finish;
            part 2: L2 + write into op_st.  Parts 0/1 may be emitted for a
            64-aligned partition slice [plo:phi] to overlap stage 1."""
            np_ = phi - plo
            e3 = e_sb[half][:].rearrange("p (c u) -> p c u", u=UP)[plo:phi]
            if part == 0:
                if plo == 0:
                    _CACHE[f"tree{half}"] = (
                        scr_pool.tile([128, UP], dt.float32, tag="emax",
                                      name=f"emax{half}"),
                        scr_pool.tile([128, T * UP], dt.float32, tag="es",
                                      name=f"es{half}"),
                        scr_pool.tile([128, M1 * 27], dt.float32, tag="s1",
                                      name=f"s1_{half}"),
                    )
                emax, es_t, s1 = _CACHE[f"tree{half}"]
                emx = emax[plo:phi]
                # emax[p,u] = max_c e[p,c,u];  es = e - emax (range <= 0)
                nc.vector.tensor_tensor(
                    emx, e3[:, 0, :], e3[:, 1, :], op=ALU.max
                )
                nc.vector.tensor_tensor(
                    emx, emx, e3[:, 2, :], op=ALU.max
                )
                emax_b = emx.unsqueeze(1).broadcast_to([np_, T, UP])
                nc.vector.tensor_tensor(
                    es_t[:].rearrange("p (c u) -> p c u", u=UP)[plo:phi], e3,
                    emax_b, op=ALU.subtract,
                )
                # L1 S-build: S[p,m,ik,j] = K[ik,j] + esA[m,j]
                es3 = es_t[:].rearrange("p (c u) -> p c u", u=UP)[plo:phi]
                esA = es3.rearrange("p c (m two) -> p m two c", two=2)
                s4 = s1[:].rearrange("p (m ik j) -> p m ik j", ik=9, j=3)[plo:phi]
                a1 = esA[:, 1:, 0, :].unsqueeze(2).broadcast_to(
                    [np_, M1 - 1, 9, 3]
                )
                nc.vector.tensor_tensor(
                    s4[:, 1:, :, :],
                    k1_v[plo:phi].unsqueeze(1).broadcast_to([np_, M1 - 1, 9, 3]),
                    a1, op=ALU.add,
                )
                a0 = esA[:, 0:1, 0, :].unsqueeze(2).broadcast_to([np_, 1, 9, 3])
                nc.vector.tensor_tensor(
                    s4[:, 0:1, :, :],
                    k0_v[plo:phi].unsqueeze(1).broadcast_to([np_, 1, 9, 3]),
                    a0, op=ALU.add,
                )
            elif part == 1:
                emax, es_t, s1 = _CACHE[f"tree{half}"]
                es3 = es_t[:].rearrange("p (c u) -> p c u", u=UP)[plo:phi]
                if plo == 0:
                    _CACHE[f"tree{half}b"] = (
                        scr_pool.tile([128, M1 * 9], dt.float32, tag="sm1",
                                      name=f"sm1_{half}"),
                        scr_pool.tile([128, M1 * 9], dt.float32, tag="c1",
                                      name=f"c1_{half}"),
                        scr_pool.tile([128, M1], dt.float32, tag="ems",
                                      name=f"ems{half}"),
                    )
                sm1, c1, emsum = _CACHE[f"tree{half}b"]
                nc.scalar.activation(s1[plo:phi], s1[plo:phi], AF.Exp)
                nc.vector.tensor_reduce(
                    sm1[plo:phi],
                    s1[:].rearrange("p (g j) -> p g j", j=3)[plo:phi],
                    axis=AX.X, op=ALU.add,
                )
                nc.scalar.activation(sm1[plo:phi], sm1[plo:phi], AF.Ln)
                # C1 = ln-sum + esB[k] + (emaxA + emaxB)  (= lnsum + eB + emaxA)
                esB = (
                    es3.rearrange("p c (m two) -> p m two c", two=2)[:, :, 1, :]
                    .unsqueeze(2)
                    .broadcast_to([np_, M1, 3, 3])
                )                                                  # [p,m,i0,k]
                c14 = c1[:].rearrange("p (m i k) -> p m i k", i=3, k=3)[plo:phi]
                nc.vector.tensor_tensor(
                    c14,
                    sm1[:].rearrange("p (m i k) -> p m i k", i=3, k=3)[plo:phi],
                    esB, op=ALU.add,
                )
                em2 = emax[:].rearrange("p (m two) -> p m two", two=2)[plo:phi]
                nc.vector.tensor_tensor(
                    emsum[plo:phi], em2[:, :, 0], em2[:, :, 1], op=ALU.add
                )
                em_b = (
                    emsum[plo:phi].unsqueeze(2).unsqueeze(3)
                    .broadcast_to([np_, M1, 3, 3])
                )
                opv = op_st[:, 144 * half : 144 * (half + 1)].rearrange(
                    "p (m i k) -> p m i k", i=3, k=3
                )[plo:phi]
                nc.vector.tensor_tensor(opv, c14, em_b, op=ALU.add)
            elif part == 2:
                # L2 pair-combine: c1 [128, 16, 9] -> op_st 8 matrices
                c1 = _CACHE[f"tree{half}b"][1]
                nm = L2M
                s_t = scr_pool.tile([128, nm * 27], dt.float32, tag="s2",
                                    name=f"s2_{half}")
                mx_t = scr_pool.tile([128, nm * 9], dt.float32, tag="mx2",
                                     name=f"mx2_{half}")
                sm_t = scr_pool.tile([128, nm * 9], dt.float32, tag="sm2",
                                     name=f"sm2_{half}")
                c_out = op_st[:, 72 * half : 72 * (half + 1)]
                cv = c1[:].rearrange("p (m two e) -> p m two e", two=2, e=9)
                b_kj = cv[:, :, 1, :].rearrange("p m (j k) -> p m k j", k=3)
                s5 = s_t[:].rearrange("p (m i k j) -> p m i k j", i=3, k=3, j=3)
                for i in range(3):
                    a_i = (
                        cv[:, :, 0, 3 * i : 3 * i + 3]
                        .unsqueeze(2)
                        .broadcast_to([128, nm, 3, 3])
                    )
                    nc.vector.tensor_tensor(
                        s5[:, :, i, :, :], a_i, b_kj, op=ALU.add
                    )
                s3 = s_t[:].rearrange("p (g j) -> p g j", j=3)
                nc.vector.tensor_reduce(mx_t[:], s3, axis=AX.X, op=ALU.max)
                mx_b = mx_t[:].unsqueeze(2).broadcast_to([128, nm * 9, 3])
                nc.vector.tensor_tensor(s3, s3, mx_b, op=ALU.subtract)
                nc.scalar.activation(s_t[:], s_t[:], AF.Exp)
                nc.vector.tensor_reduce(sm_t[:], s3, axis=AX.X, op=ALU.add)
                nc.scalar.activation(sm_t[:], sm_t[:], AF.Ln)
                nc.vector.tensor_tensor(c_out, sm_t[:], mx_t[:], op=ALU.add)

        def emit_numerator(half, plo=0, phi=128):
            yoh = cf_sb[:, 54 + T * UP * half : 54 + T * UP * (half + 1)]
            if plo == 0:
                _CACHE[f"nsc{half}"] = scr_pool.tile(
                    [128, T * UP], dt.float32, tag="nsc", name=f"nsc{half}"
                )
            scr = _CACHE[f"nsc{half}"]
            nc.vector.tensor_tensor(
                scr[plo:phi], e_sb[half][plo:phi], yoh[plo:phi], op=ALU.mult
            )
            nc.vector.tensor_reduce(
                op_st[plo:phi, 288 + half : 289 + half], scr[plo:phi],
                axis=AX.X, op=ALU.add,
            )

        # ---- stage 1 + interleaved tree emission ----
        # block list: single-group blocks at the edges for lower pipeline
        # latency, 2-group blocks in the middle for best queue throughput
        BLOCKS = [(0,), (1,), (2, 3), (4, 5), (6, 7), (8, 9), (10, 11),
                  (12, 13), (14,), (15,)]
        first_of = {blk[0]: (i, blk) for i, blk in enumerate(BLOCKS)}

        def issue_block(ib):
            blk = BLOCKS[ib]
            n = len(blk)
            xg = xg_pool.tile([128, HC * 512 * n], dt.bfloat16, tag="xg",
                              name=f"xgb{ib % 4}_{n}")
            xq = nc.sync if ib % 2 == 0 else nc.scalar
            xq.dma_start(
                xg[:].rearrange("p (j t) -> p j t", t=512 * n),
                xt_d[:]
                .rearrange("(j p) t -> p j t", p=128)[:, :, 512 * blk[0] :
                                                     512 * (blk[-1] + 1)],
            )
            return xg

        xgs = {}
        xgs[0] = issue_block(0)
        xgs[1] = issue_block(1)
        nc.sync.dma_start(wt_sb[:], wt_d[:])
        nc.sync.dma_start(cf_sb[:], cf_d[:])
        xgs[2] = issue_block(2)
        xgs[3] = issue_block(3)
        # PE warm-up during the startup window: keeps the tensor engine's
        # DVFS ramped so real matmuls run at full clock from the first block
        wu_ps = ps_e_pool.tile([T, 512], dt.float32, tag="wups", bufs=1)
        for w in range(16):
            nc.tensor.matmul(
                wu_ps[:],
                wt_sb[:, 0:T],
                wt_sb[:, 0:1].broadcast_to([128, 512]),
                start=(w == 0),
                stop=(w == 15),
            )
        for g in range(NGROUP):
            if g in first_of:
                ib, blk = first_of[g]
                if ib + 4 < len(BLOCKS):
                    xgs[ib + 4] = issue_block(ib + 4)
                xg = xgs.pop(ib)
                goff = g
            e_ps = ps_e_pool.tile([T, 512], dt.float32, tag="eps")
            sg = g - goff
            nblk = xg.shape[1] // (HC * 512)
            for j in range(HC):
                nc.tensor.matmul(
                    e_ps[:],
                    wt_sb[:, T * j : T * (j + 1)],
                    xg[:, 512 * nblk * j + 512 * sg : 512 * nblk * j
                       + 512 * (sg + 1)],
                    start=(j == 0),
                    stop=(j == HC - 1),
                )
            if g % 4 == 0:
                e_stage = st_pool.tile([T, 2048], dt.float32, tag="estage")
            nc.scalar.activation(
                e_stage[:, 512 * (g % 4) : 512 * (g % 4 + 1)], e_ps[:], AF.Copy
            )
            if g in (3, 7, 11):
                # redistribute 4 groups (2048 t = 64 rows of 32 steps)
                h, q = g // 8, (g // 4) % 2
                for c in range(T):
                    nc.sync.dma_start(
                        e_sb[h][:].rearrange("p (c u) -> p c u", u=UP)[
                            64 * q : 64 * (q + 1), c
                        ],
                        e_stage[c : c + 1, :].rearrange("z (r u) -> z r u", u=UP),
                    )
            elif g in (13, 15):
                # last batch redistributes in 2-group chunks so the final
                # tree slices can start earlier
                r0 = 64 + 32 * ((g - 13) // 2)
                co = 1024 * ((g - 13) // 2)
                for c in range(T):
                    nc.sync.dma_start(
                        e_sb[1][:].rearrange("p (c u) -> p c u", u=UP)[
                            r0 : r0 + 32, c
                        ],
                        e_stage[c : c + 1, co : co + 1024].rearrange(
                            "z (r u) -> z r u", u=UP
                        ),
                    )
            # interleave trees: all of half 0 and the first 64 partitions of
            # half 1 hide under stage 1; the tail is only L1[64:128] + L2
            if g == 8:
                emit_tree(0, 0, 0, 64)
                emit_tree(0, 1, 0, 64)
            elif g == 9:
                emit_tree(0, 0, 64, 128)
                emit_tree(0, 1, 64, 128)
            elif g == 11:
                emit_numerator(0)
            elif g == 12:
                emit_tree(1, 0, 0, 64)
            elif g == 13:
                emit_tree(1, 1, 0, 64)
            elif g == 14:
                emit_tree(1, 0, 64, 96)
                emit_numerator(1, 0, 64)
            elif g == 15:
                emit_tree(1, 1, 64, 96)

        # ---- tail: second-half tree remainder + numerator + output ----
        emit_tree(1, 0, 96, 128)
        emit_tree(1, 1, 96, 128)
        emit_numerator(1, 64, 128)
        nc.sync.dma_start(op_d[:], op_st[:])

    return nc


def _get_program():
    if "nc" not in _CACHE:
        _CACHE["nc"] = _build_program()
    return _CACHE["nc"]


def kernel(x, y, mask, W, b, start_transitions, end_transitions, transitions):
    x = np.asarray(x, dtype=np.float32)
    y = np.asarray(y, dtype=np.int32)
    W = np.asarray(W, dtype=np.float32)
    b = np.asarray(b, dtype=np.float32)
    start_t = np.asarray(start_transitions, dtype=np.float32)
    end_t = np.asarray(end_transitions, dtype=np.float32)
    trans = np.asarray(transitions, dtype=np.float32)

    nc = _get_program()

    # ---- host-prepared constants ----
    wt = np.zeros((128, HC * T), dtype=np.float32)
    for j in range(HC):
        for c in range(T):
            wt[:, T * j + c] = W[c, 128 * j : 128 * (j + 1)]

    ct = trans + b[None, :]                      # ct[i,j] = trans[i,j]+b[j]
    k1 = np.empty((3, 3, 3), dtype=np.float32)   # k1[i,k,j] = ct[i,j]+ct[j,k]
    k0 = np.empty((3, 3, 3), dtype=np.float32)   # alpha0 row: start[j]+b[j]+ct[j,k]
    sb = start_t + b
    for i in range(3):
        for k in range(3):
            for j in range(3):
                k1[i, k, j] = ct[i, j] + ct[j, k]
                k0[i, k, j] = sb[j] + ct[j, k]
    cf_base = np.zeros((128, 54), dtype=np.float32)
    cf_base[:, 0:27] = k1.reshape(27)[None, :]
    cf_base[:, 27:54] = k1.reshape(27)[None, :]
    cf_base[0, 27:54] = k0.reshape(27)

    in_maps = []
    for core in range(NCORES):
        b0 = BL * core
        yc = y[b0 : b0 + BL].reshape(2, 128, UP)           # (h, p, u)
        yoh = np.zeros((128, 2, T, UP), dtype=np.float32)  # (p, h, c, u)
        for c in range(T):
            yoh[:, :, c, :] = (yc == c).transpose(1, 0, 2)
        cf = np.concatenate([cf_base, yoh.reshape(128, 2 * T * UP)], axis=1)
        im = {
            "xt": np.ascontiguousarray(
                x[b0 : b0 + BL].reshape(NT, H).T
            ).astype(ml_dtypes.bfloat16),
            "wt": wt.astype(ml_dtypes.bfloat16),
            "cf": np.ascontiguousarray(cf),
        }
        in_maps.append(im)

    _CACHE["last_in_maps"] = in_maps
    res = run_bass_kernel_spmd(nc, in_maps, core_ids=list(range(NCORES)))
    results = res.results

    # ---- host epilogue ----
    chains = np.empty((B, 128 * M1, 3, 3), dtype=np.float64)
    gsum = np.empty(B, dtype=np.float64)
    for core in range(NCORES):
        op = np.asarray(results[core]["op"], dtype=np.float64)  # [128, 290]
        for h in range(BL):
            bidx = BL * core + h
            chains[bidx] = op[:, 144 * h : 144 * (h + 1)].reshape(128 * M1, 3, 3)
            gsum[bidx] = op[:, 288 + h].sum()

    # vectorized log-semiring product over the chain (float64)
    cur = chains
    while cur.shape[1] > 1:
        A = cur[:, 0::2]                                   # [B, n, 3, 3] (i,j)
        Bm = cur[:, 1::2]                                  # [B, n, 3, 3] (j,k)
        s = A[:, :, :, :, None] + Bm[:, :, None, :, :]     # [B, n, i, j, k]
        m = s.max(axis=3)
        cur = m + np.log(np.exp(s - m[:, :, :, None, :]).sum(axis=3))
    P = cur[:, 0]                                          # [B, 3, 3]

    losses = np.zeros(B, dtype=np.float64)
    for bidx in range(B):
        yb = y[bidx]
        az = P[bidx, 0, :] + end_t.astype(np.float64)
        mz = az.max()
        denom = mz + np.log(np.exp(az - mz).sum())
        num = (
            start_t[yb[0]]
            + gsum[bidx]
            + b[yb].sum()                     # bias not in device emissions
            + trans[yb[:-1], yb[1:]].sum()
            + end_t[yb[-1]]
        )
        losses[bidx] = num - denom
    return np.float32(-np.mean(losses))
